# revision 15
# baseline (speedup 1.0000x reference)
"""Multi-head attention (B=2, T=2048, D=2048, H=16, HD=128) on 8 Trainium2
NeuronCores.

Sharding: core c in 0..7 handles batch b = c // 4 and head group g = c % 4
(4 heads per core) — tensor-parallel over heads within each batch element.
wq/wk/wv are column-sharded (rows of the (D,D) weight, since y = x @ W.T),
wo is row-sharded; the partial outputs (one per head group) are summed on
the host (the "all-reduce"), then the two batch elements are stacked.

Device kernel (per core, SPMD):
  phase A1: KT (roped) and V projections, streaming xT in t-quarters that
            stay resident in SBUF (bf16)
  phase A2: QT (roped, pre-scaled) projection from the held x tiles (no
            second x DMA)
  phase B:  per (q-chunk, head): scoresT = KT_k-tile.T @ QT (k on partitions,
            q on free dim), exp on ACT (no max subtraction — scores are
            O(5) so exp is safe in fp32), unnormalized out accumulated as
            V.T-matmul with exp(scores) as the moving operand (no PE
            transposes anywhere), softmax denominators via ones-matmul,
            normalization via a K=1 broadcast matmul + DVE multiply
  phase C:  per q-chunk: partial_y = aoT.T @ woT accumulated over the 4
            head k-steps, DMA'd out per (t-tile, e-chunk)

All matmul operands are bfloat16 (1 cycle/row on the PE like fp32r, but
half the DMA/SBUF footprint and fast-weight-load eligible); accumulation
stays fp32 in PSUM.  RoPE pairs are made partition-contiguous by permuting
the wq/wk output rows per head on the host (even hd components land in
partitions 0..63, odd in 64..127), which turns the rotation into four
full-width DVE ops against host-precomputed [cos;cos] and [-sin;sin]
tables. The softmax scale is folded into wq. The partial output py is
returned in bf16 and summed across head-group cores in fp32 on the host.
"""
from contextlib import ExitStack

import numpy as np

B, T, D, H = 2, 2048, 2048, 16
HD = D // H            # 128
N_CORES = 8
HPC = H // 4           # 4 heads per core
JC = HPC * HD          # 512 per-core projection width
KT_TILES = T // 128    # 16 k tiles
QC = 512               # q-chunk width in phase B
N_QC = T // QC         # 4
TE = 512               # t-quarter width in phase A
N_TE = T // TE         # 4
KD = D // 128          # 16 contraction tiles for the projections

import os as _os

SC_BUFS = int(_os.environ.get("K_SC_BUFS", "2"))
# analysis aid: truncate the program after phase a1 / a2 / b (default: full)
PHASES = _os.environ.get("K_PHASES", "full")
PSA_BUFS = int(_os.environ.get("K_PSA_BUFS", "4"))
RT_BUFS = int(_os.environ.get("K_RT_BUFS", "3"))
PT_BUFS = int(_os.environ.get("K_PT_BUFS", "2"))
XSPLIT = int(_os.environ.get("K_XSPLIT", "4"))    # xte DMA chunks

_cache = {}


def _build_program():
    import concourse.bacc as bacc
    import concourse.tile as tile
    from concourse import mybir

    F32 = mybir.dt.float32
    F32R = mybir.dt.float32r
    BF16 = mybir.dt.bfloat16
    AF = mybir.ActivationFunctionType
    ALU = mybir.AluOpType

    nc = bacc.Bacc("TRN2", target_bir_lowering=False, debug=False,
                   num_devices=N_CORES)

    # All big inputs are pre-tiled on the host into the exact SBUF layout
    # ([128, free]) so every DMA is a dense contiguous copy with 2KB+ lines.
    xS = nc.dram_tensor("xS", [128, N_TE * KD * TE], BF16,
                        kind="ExternalInput").ap()
    wqS = nc.dram_tensor("wqS", [128, KD * JC], BF16,
                         kind="ExternalInput").ap()
    wkS = nc.dram_tensor("wkS", [128, KD * JC], BF16,
                         kind="ExternalInput").ap()
    wvS = nc.dram_tensor("wvS", [128, KD * JC], BF16,
                         kind="ExternalInput").ap()
    woS = nc.dram_tensor("woS", [128, HPC * D], BF16,
                         kind="ExternalInput").ap()
    csA = nc.dram_tensor("csA", [128, T], F32, kind="ExternalInput").ap()
    csB = nc.dram_tensor("csB", [128, T], F32, kind="ExternalInput").ap()
    ones1 = nc.dram_tensor("ones1", [128, 1], BF16, kind="ExternalInput").ap()
    ones2 = nc.dram_tensor("ones2", [1, 128], F32R, kind="ExternalInput").ap()
    py = nc.dram_tensor("py", [T, D], BF16, kind="ExternalOutput").ap()

    with tile.TileContext(nc) as tc, ExitStack() as ctx:
        # long-lived pools on the RIGHT side of the SBUF heap (the tile
        # allocator is a per-side LIFO stack; phase-scoped pools live on the
        # default left side and can come and go underneath these)
        p_qkv = ctx.enter_context(tc.tile_pool(name="qkv", bufs=1,
                                               side="right"))

        KT = [p_qkv.tile([128, T], BF16, tag=f"KT{h}", name=f"KT{h}")
              for h in range(HPC)]
        V = [p_qkv.tile([128, JC], BF16, tag=f"V{t}", name=f"V{t}")
             for t in range(KT_TILES)]
        QT = [p_qkv.tile([128, T], BF16, tag=f"QT{h}", name=f"QT{h}")
              for h in range(HPC)]

        def rope(ps_tile, dst, t0, tw, pool_tmp):
            """dst[:, t0:t0+tw] = rotate(ps_tile) using csA/csB tables."""
            u = pool_tmp.tile([128, tw], F32, tag="ropeu")
            v = pool_tmp.tile([128, tw], F32, tag="ropev")
            nc.vector.tensor_tensor(u[:], ps_tile[:], csa_t[:, t0:t0 + tw],
                                    ALU.mult)
            nc.vector.tensor_tensor(v[0:64, :], ps_tile[64:128, :],
                                    csb_t[0:64, t0:t0 + tw], ALU.mult)
            nc.vector.tensor_tensor(v[64:128, :], ps_tile[0:64, :],
                                    csb_t[64:128, t0:t0 + tw], ALU.mult)
            nc.vector.tensor_tensor(dst[:, t0:t0 + tw], u[:], v[:], ALU.add)

        # ---- phase A: projections ----
        with tc.tile_pool(name="cs", bufs=1) as p_cs:
            csa_t = p_cs.tile([128, T], F32, tag="csa")
            csb_t = p_cs.tile([128, T], F32, tag="csb")

            with tc.tile_pool(name="xa", bufs=1) as p_x, \
                 tc.tile_pool(name="ropetmp", bufs=RT_BUFS) as p_rt, \
                 tc.tile_pool(name="psA", bufs=PSA_BUFS, space="PSUM") as psA:

                def load_xte(e, split=1):
                    # split per k-group so the first matmul of the quarter
                    # waits on a fraction of the 2MB, not all of it
                    xte = p_x.tile([128, KD * TE], BF16, tag=f"xte{e}",
                                   name=f"xte{e}")
                    kc = KD // split
                    base = e * KD * TE
                    for k4 in range(0, KD, kc):
                        nc.sync.dma_start(
                            xte[:, k4 * TE:(k4 + kc) * TE],
                            xS[:, base + k4 * TE:base + (k4 + kc) * TE])
                    return xte

                def load_w(pool, dram, tag, split):
                    # per-k-chunk DMAs: first projection matmul only waits
                    # for its own k slice instead of the full weight
                    wt = pool.tile([128, KD * JC], BF16, tag=tag, name=tag)
                    kc = KD // split
                    for k in range(0, KD, kc):
                        nc.sync.dma_start(
                            wt[:, k * JC:(k + kc) * JC],
                            dram[:, k * JC:(k + kc) * JC])
                    return wt

                def proj_qk(wt, xte, e, dst):
                    # dst[j][:, eslice] = rope((w x)^T)
                    for j in range(HPC):
                        acc = psA.tile([128, TE], F32, tag="qk")
                        for k in range(KD):
                            nc.tensor.matmul(
                                acc[:],
                                wt[:, k * JC + j * 128:k * JC + (j + 1) * 128],
                                xte[:, k * TE:(k + 1) * TE],
                                start=(k == 0), stop=(k == KD - 1),
                            )
                        rope(acc, dst[j], e * TE, TE, p_rt)

                def proj_v(xte, e):
                    for tl in range(TE // 128):
                        tt = e * (TE // 128) + tl
                        acc = psA.tile([128, JC], F32, tag="v", name="acc")
                        for k in range(KD):
                            nc.tensor.matmul(
                                acc[:],
                                xte[:, k * TE + tl * 128:
                                    k * TE + (tl + 1) * 128],
                                wv_t[:, k * JC:(k + 1) * JC],
                                start=(k == 0), stop=(k == KD - 1),
                            )
                        nc.vector.tensor_copy(V[tt][:], acc[:])

                # A1: K and V (wk, wv resident). Emission order matters:
                # the DMA pipe drains roughly in order, so interleave the wk
                # chunks with the first x quarter (K(q0) consumes both in k
                # order), then cs (first rope needs it ~13us in), then wv
                # (first V proj ~17us in), then the remaining x quarters.
                with tc.tile_pool(name="wkv", bufs=1) as p_w:
                    xtiles = [None] * N_TE
                    wk_t = p_w.tile([128, KD * JC], BF16, tag="wk", name="wk")
                    xte0 = p_x.tile([128, KD * TE], BF16, tag="xte0",
                                    name="xte0")
                    xtiles[0] = xte0
                    kc = KD // 4
                    for k4 in range(0, KD, kc):
                        nc.sync.dma_start(
                            wk_t[:, k4 * JC:(k4 + kc) * JC],
                            wkS[:, k4 * JC:(k4 + kc) * JC])
                        nc.sync.dma_start(
                            xte0[:, k4 * TE:(k4 + kc) * TE],
                            xS[:, k4 * TE:(k4 + kc) * TE])
                    nc.sync.dma_start(csa_t[:], csA[:])
                    nc.sync.dma_start(csb_t[:], csB[:])
                    wv_t = load_w(p_w, wvS, "wv", 4)
                    xtiles[1] = load_xte(1)
                    xtiles[2] = load_xte(2)
                    xtiles[3] = load_xte(3)

                    for e in range(N_TE):
                        proj_qk(wk_t, xtiles[e], e, KT)
                        proj_v(xtiles[e], e)

                # A2: Q (wq resident) on the held x tiles
                if PHASES != "a1":
                    with tc.tile_pool(name="wq", bufs=1) as p_w:
                        wq_t = load_w(p_w, wqS, "wq", 4)
                        for e in range(N_TE):
                            proj_qk(wq_t, xtiles[e], e, QT)

        # ---- phases B + C ----
        if PHASES not in ("a1", "a2"):
            with tc.tile_pool(name="wo", bufs=1) as p_wo, \
                 tc.tile_pool(name="pt", bufs=PT_BUFS) as p_pt, \
                 tc.tile_pool(name="ao", bufs=6) as p_ao, \
                 tc.tile_pool(name="bmisc", bufs=2) as p_bm, \
                 tc.tile_pool(name="pyout", bufs=int(_os.environ.get("K_PYO", "4"))) as p_po, \
                 tc.tile_pool(name="psSC", bufs=SC_BUFS, space="PSUM") as psSC, \
                 tc.tile_pool(name="psOU", bufs=int(_os.environ.get("K_OU_BUFS", "2")), space="PSUM") as psOU, \
                 tc.tile_pool(name="psSM", bufs=1, space="PSUM") as psSM, \
                 tc.tile_pool(name="psBC", bufs=1, space="PSUM") as psBC, \
                 tc.tile_pool(name="psC", bufs=int(_os.environ.get("K_PY_BUFS", "2")), space="PSUM") as psC:

                wo_t = p_wo.tile([128, HPC * D], BF16, tag="wo")
                nc.sync.dma_start(wo_t[:], woS[:])
                o1_t = p_bm.tile([128, 1], BF16, tag="o1")
                o2_t = p_bm.tile([1, 128], F32R, tag="o2")
                nc.sync.dma_start(o1_t[:], ones1[:])
                nc.sync.dma_start(o2_t[:], ones2[:])

                for qc in range(N_QC):
                    qs = qc * QC
                    ao = []
                    for h in range(HPC):
                        pt = p_pt.tile([128, KT_TILES * QC], BF16, tag="pt")
                        for k in range(KT_TILES):
                            sc = psSC.tile([128, QC], F32, tag="sc")
                            nc.tensor.matmul(
                                sc[:],
                                KT[h][:, k * 128:(k + 1) * 128],
                                QT[h][:, qs:qs + QC],
                                start=True, stop=True,
                            )
                            nc.scalar.activation(
                                pt[:, k * QC:(k + 1) * QC], sc[:], AF.Exp)
                        ou = psOU.tile([128, QC], F32, tag="ou")
                        sm = psSM.tile([1, QC], F32, tag="sm")
                        for k in range(KT_TILES):
                            nc.tensor.matmul(
                                ou[:],
                                V[k][:, h * 128:(h + 1) * 128],
                                pt[:, k * QC:(k + 1) * QC],
                                start=(k == 0), stop=(k == KT_TILES - 1),
                            )
                            nc.tensor.matmul(
                                sm[:], o1_t[:], pt[:, k * QC:(k + 1) * QC],
                                start=(k == 0), stop=(k == KT_TILES - 1),
                            )
                        rc = p_bm.tile([1, QC], F32R, tag="rc")
                        with nc.allow_low_precision(reason="softmax denom in tf32"):
                            nc.vector.reciprocal(rc[:], sm[:])
                        bc = psBC.tile([128, QC], F32, tag="bc")
                        nc.tensor.matmul(bc[:], o2_t[:], rc[:],
                                         start=True, stop=True)
                        # TT cannot read two PSUM operands; stage bc in SBUF
                        bc_sb = p_bm.tile([128, QC], F32, tag="bcsb")
                        nc.vector.tensor_copy(bc_sb[:], bc[:])
                        ao_h = p_ao.tile([128, QC], BF16, tag="ao")
                        nc.vector.tensor_tensor(ao_h[:], ou[:], bc_sb[:], ALU.mult)
                        ao.append(ao_h)

                    # phase C for this q-chunk
                    if PHASES == "b":
                        continue
                    for tl in range(QC // 128):
                        ts = qs + tl * 128
                        out_sb = p_po.tile([128, D], BF16, tag="pyo")
                        for ec in range(D // 512):
                            acc = psC.tile([128, 512], F32, tag="py")
                            for j in range(HPC):
                                nc.tensor.matmul(
                                    acc[:],
                                    ao[j][:, tl * 128:(tl + 1) * 128],
                                    wo_t[:, j * D + ec * 512:j * D + (ec + 1) * 512],
                                    start=(j == 0), stop=(j == HPC - 1),
                                )
                            nc.vector.tensor_copy(
                                out_sb[:, ec * 512:(ec + 1) * 512], acc[:])
                        nc.sync.dma_start(py[ts:ts + 128, :], out_sb[:])

    nc.compile()
    return nc


def _prep_inputs(x, freqs_cis, wq, wk, wv, wo):
    """Host-side shard + layout prep. Returns in_maps for the 8 cores."""
    import ml_dtypes

    BF = ml_dtypes.bfloat16
    scale = HD ** (-0.5)
    # even/odd permutation within each head's 128 rows
    perm = np.concatenate([np.arange(0, HD, 2), np.arange(1, HD, 2)])

    cos = np.ascontiguousarray(freqs_cis[:, :, 0].T, dtype=np.float32)  # (64,T)
    sin = np.ascontiguousarray(freqs_cis[:, :, 1].T, dtype=np.float32)
    csA = np.concatenate([cos, cos], axis=0)          # (128, T)
    csB = np.concatenate([-sin, sin], axis=0)         # (128, T)
    ones1 = np.ones((128, 1), BF)
    ones2 = np.ones((1, 128), np.float32)

    def tile_w(wT):
        # (D, JC) -> SBUF layout [128, KD*JC]: row p, col k*JC+j = wT[k*128+p, j]
        return np.ascontiguousarray(
            wT.reshape(KD, 128, JC).transpose(1, 0, 2).reshape(128, KD * JC)
        ).astype(BF)

    in_maps = []
    for c in range(N_CORES):
        b, g = divmod(c, 4)
        rows = slice(g * JC, (g + 1) * JC)
        wq_g = wq[rows].reshape(HPC, HD, D)[:, perm].reshape(JC, D) * scale
        wk_g = wk[rows].reshape(HPC, HD, D)[:, perm].reshape(JC, D)
        wv_g = wv[rows]
        wo_g = wo[:, rows]
        # x: [128, N_TE*KD*TE]: row p, col (e*KD+k)*TE+t = x[b][e*TE+t, k*128+p]
        xS = np.ascontiguousarray(
            x[b].reshape(N_TE, TE, KD, 128).transpose(3, 0, 2, 1)
            .reshape(128, N_TE * KD * TE)).astype(BF)
        # wo: [128, HPC*D]: row p, col j*D+e = wo_g.T[j*128+p, e]
        woS = np.ascontiguousarray(
            np.ascontiguousarray(wo_g.T).reshape(HPC, 128, D)
            .transpose(1, 0, 2).reshape(128, HPC * D)).astype(BF)
        in_maps.append({
            "xS": xS,
            "wqS": tile_w(np.ascontiguousarray(wq_g.T)),
            "wkS": tile_w(np.ascontiguousarray(wk_g.T)),
            "wvS": tile_w(np.ascontiguousarray(wv_g.T)),
            "woS": woS,
            "csA": csA,
            "csB": csB,
            "ones1": ones1,
            "ones2": ones2,
        })
    return in_maps


def _make_runner(nc):
    """Cacheable jitted SPMD runner (mirrors bass2jax.run_bass_via_pjrt's
    multi-core path, minus donation, so one jit serves repeated calls)."""
    import jax
    from concourse import mybir
    from concourse.bass2jax import (
        _bass_exec_p, install_neuronx_cc_hook, partition_id_tensor)
    from jax.experimental.shard_map import shard_map
    from jax.sharding import Mesh, NamedSharding, PartitionSpec

    install_neuronx_cc_hook()
    partition_name = (
        nc.partition_id_tensor.name if nc.partition_id_tensor else None)
    in_names, out_names, out_avals, zero_outs = [], [], [], []
    for alloc in nc.m.functions[0].allocations:
        if not isinstance(alloc, mybir.MemoryLocationSet):
            continue
        name = alloc.memorylocations[0].name
        if alloc.kind == "ExternalInput":
            if name != partition_name:
                in_names.append(name)
        elif alloc.kind == "ExternalOutput":
            out_names.append(name)
            shape = tuple(alloc.tensor_shape)
            dtype = mybir.dt.np(alloc.dtype)
            out_avals.append(jax.core.ShapedArray(shape, dtype))
            zero_outs.append(np.zeros(shape, dtype))
    all_in_names = list(in_names) + out_names
    if partition_name is not None:
        all_in_names.append(partition_name)

    def _body(*args):
        operands = list(args)
        if partition_name is not None:
            operands.append(partition_id_tensor())
        outs = _bass_exec_p.bind(
            *operands,
            out_avals=tuple(out_avals),
            in_names=tuple(all_in_names),
            out_names=tuple(out_names),
            lowering_input_output_aliases=(),
            sim_require_finite=True,
            sim_require_nnan=True,
            nc=nc,
        )
        return tuple(outs)

    devices = jax.devices()[:N_CORES]
    assert len(devices) == N_CORES, f"need {N_CORES} devices, got {devices}"
    mesh = Mesh(np.asarray(devices), ("core",))
    nshard = NamedSharding(mesh, PartitionSpec("core"))
    n_in = len(in_names) + len(out_names)
    jf = jax.jit(
        shard_map(_body, mesh=mesh,
                  in_specs=(PartitionSpec("core"),) * n_in,
                  out_specs=(PartitionSpec("core"),) * len(out_names),
                  check_rep=False),
        keep_unused=True,
    )
    dev_zero = [
        jax.device_put(
            np.zeros((N_CORES * z.shape[0], *z.shape[1:]), z.dtype), nshard)
        for z in zero_outs
    ]

    def run(in_maps):
        concat_in = [
            np.concatenate([np.asarray(in_maps[c][nm])
                            for c in range(N_CORES)], axis=0)
            for nm in in_names
        ]
        dev_in = [jax.device_put(a, nshard) for a in concat_in]
        outs = jf(*dev_in, *dev_zero)
        return {
            name: np.asarray(outs[i]) for i, name in enumerate(out_names)
        }

    return run


def kernel(x, freqs_cis, wq, wk, wv, wo):
    if "nc" not in _cache:
        _cache["nc"] = _build_program()
    if "run" not in _cache:
        _cache["run"] = _make_runner(_cache["nc"])

    in_maps = _prep_inputs(
        np.asarray(x), np.asarray(freqs_cis), np.asarray(wq),
        np.asarray(wk), np.asarray(wv), np.asarray(wo))
    outs = _cache["run"](in_maps)
    pys = outs["py"].astype(np.float32).reshape(N_CORES, T, D)

    out = np.empty((B, T, D), dtype=np.float32)
    for b in range(B):
        acc = pys[b * 4]
        for g in range(1, 4):
            acc = acc + pys[b * 4 + g]
        out[b] = acc
    return out


# revision 16
# speedup vs baseline: 1.0113x; 1.0113x over previous
"""Multi-head attention (B=2, T=2048, D=2048, H=16, HD=128) on 8 Trainium2
NeuronCores.

Sharding: core c in 0..7 handles batch b = c // 4 and head group g = c % 4
(4 heads per core) — tensor-parallel over heads within each batch element.
wq/wk/wv are column-sharded (rows of the (D,D) weight, since y = x @ W.T),
wo is row-sharded; the partial outputs (one per head group) are summed on
the host (the "all-reduce"), then the two batch elements are stacked.

Device kernel (per core, SPMD):
  phase A1: KT (roped) and V projections, streaming xT in t-quarters that
            stay resident in SBUF (bf16)
  phase A2: QT (roped, pre-scaled) projection from the held x tiles (no
            second x DMA)
  phase B:  per (q-chunk, head): scoresT = KT_k-tile.T @ QT (k on partitions,
            q on free dim), exp on ACT (no max subtraction — scores are
            O(5) so exp is safe in fp32), unnormalized out accumulated as
            V.T-matmul with exp(scores) as the moving operand (no PE
            transposes anywhere), softmax denominators via ones-matmul,
            normalization via a K=1 broadcast matmul + DVE multiply
  phase C:  per q-chunk: partial_y = aoT.T @ woT accumulated over the 4
            head k-steps, DMA'd out per (t-tile, e-chunk)

All matmul operands are bfloat16 (1 cycle/row on the PE like fp32r, but
half the DMA/SBUF footprint and fast-weight-load eligible); accumulation
stays fp32 in PSUM.  RoPE pairs are made partition-contiguous by permuting
the wq/wk output rows per head on the host (even hd components land in
partitions 0..63, odd in 64..127), which turns the rotation into four
full-width DVE ops against host-precomputed [cos;cos] and [-sin;sin]
tables. The softmax scale is folded into wq. The partial output py is
returned in bf16 and summed across head-group cores in fp32 on the host.
"""
from contextlib import ExitStack

import numpy as np

B, T, D, H = 2, 2048, 2048, 16
HD = D // H            # 128
N_CORES = 8
HPC = H // 4           # 4 heads per core
JC = HPC * HD          # 512 per-core projection width
KT_TILES = T // 128    # 16 k tiles
QC = 512               # q-chunk width in phase B
N_QC = T // QC         # 4
TE = 512               # t-quarter width in phase A
N_TE = T // TE         # 4
KD = D // 128          # 16 contraction tiles for the projections

import os as _os

SC_BUFS = int(_os.environ.get("K_SC_BUFS", "2"))
# analysis aid: truncate the program after phase a1 / a2 / b (default: full)
PHASES = _os.environ.get("K_PHASES", "full")
PSA_BUFS = int(_os.environ.get("K_PSA_BUFS", "4"))
RT_BUFS = int(_os.environ.get("K_RT_BUFS", "3"))
PT_BUFS = int(_os.environ.get("K_PT_BUFS", "2"))
XSPLIT = int(_os.environ.get("K_XSPLIT", "4"))    # xte DMA chunks

_cache = {}


def _build_program():
    import concourse.bacc as bacc
    import concourse.tile as tile
    from concourse import mybir

    F32 = mybir.dt.float32
    F32R = mybir.dt.float32r
    BF16 = mybir.dt.bfloat16
    AF = mybir.ActivationFunctionType
    ALU = mybir.AluOpType

    nc = bacc.Bacc("TRN2", target_bir_lowering=False, debug=False,
                   num_devices=N_CORES)

    # All big inputs are pre-tiled on the host into the exact SBUF layout
    # ([128, free]) so every DMA is a dense contiguous copy with 2KB+ lines.
    xS = nc.dram_tensor("xS", [128, N_TE * KD * TE], BF16,
                        kind="ExternalInput").ap()
    wqS = nc.dram_tensor("wqS", [128, KD * JC], BF16,
                         kind="ExternalInput").ap()
    wkS = nc.dram_tensor("wkS", [128, KD * JC], BF16,
                         kind="ExternalInput").ap()
    wvS = nc.dram_tensor("wvS", [128, KD * JC], BF16,
                         kind="ExternalInput").ap()
    woS = nc.dram_tensor("woS", [128, HPC * D], BF16,
                         kind="ExternalInput").ap()
    csA = nc.dram_tensor("csA", [128, T], F32, kind="ExternalInput").ap()
    csB = nc.dram_tensor("csB", [128, T], F32, kind="ExternalInput").ap()
    ones1 = nc.dram_tensor("ones1", [128, 1], BF16, kind="ExternalInput").ap()
    ones2 = nc.dram_tensor("ones2", [1, 128], F32R, kind="ExternalInput").ap()
    py = nc.dram_tensor("py", [T, D], BF16, kind="ExternalOutput").ap()

    with tile.TileContext(nc) as tc, ExitStack() as ctx:
        # long-lived pools on the RIGHT side of the SBUF heap (the tile
        # allocator is a per-side LIFO stack; phase-scoped pools live on the
        # default left side and can come and go underneath these)
        p_qkv = ctx.enter_context(tc.tile_pool(name="qkv", bufs=1,
                                               side="right"))

        KT = [p_qkv.tile([128, T], BF16, tag=f"KT{h}", name=f"KT{h}")
              for h in range(HPC)]
        V = [p_qkv.tile([128, JC], BF16, tag=f"V{t}", name=f"V{t}")
             for t in range(KT_TILES)]
        QT = [p_qkv.tile([128, T], BF16, tag=f"QT{h}", name=f"QT{h}")
              for h in range(HPC)]

        def rope(ps_tile, dst, t0, tw, pool_tmp):
            """dst[:, t0:t0+tw] = rotate(ps_tile) using csA/csB tables."""
            u = pool_tmp.tile([128, tw], F32, tag="ropeu")
            v = pool_tmp.tile([128, tw], F32, tag="ropev")
            nc.vector.tensor_tensor(u[:], ps_tile[:], csa_t[:, t0:t0 + tw],
                                    ALU.mult)
            nc.vector.tensor_tensor(v[0:64, :], ps_tile[64:128, :],
                                    csb_t[0:64, t0:t0 + tw], ALU.mult)
            nc.vector.tensor_tensor(v[64:128, :], ps_tile[0:64, :],
                                    csb_t[64:128, t0:t0 + tw], ALU.mult)
            nc.vector.tensor_tensor(dst[:, t0:t0 + tw], u[:], v[:], ALU.add)

        # ---- phase A: projections ----
        with tc.tile_pool(name="cs", bufs=1) as p_cs:
            csa_t = p_cs.tile([128, T], F32, tag="csa")
            csb_t = p_cs.tile([128, T], F32, tag="csb")

            with tc.tile_pool(name="xa", bufs=1) as p_x, \
                 tc.tile_pool(name="ropetmp", bufs=RT_BUFS) as p_rt, \
                 tc.tile_pool(name="psA", bufs=PSA_BUFS, space="PSUM") as psA:

                def load_xte(e, split=1):
                    # split per k-group so the first matmul of the quarter
                    # waits on a fraction of the 2MB, not all of it
                    xte = p_x.tile([128, KD * TE], BF16, tag=f"xte{e}",
                                   name=f"xte{e}")
                    kc = KD // split
                    base = e * KD * TE
                    for k4 in range(0, KD, kc):
                        nc.sync.dma_start(
                            xte[:, k4 * TE:(k4 + kc) * TE],
                            xS[:, base + k4 * TE:base + (k4 + kc) * TE])
                    return xte

                def load_w(pool, dram, tag, split):
                    # per-k-chunk DMAs: first projection matmul only waits
                    # for its own k slice instead of the full weight
                    wt = pool.tile([128, KD * JC], BF16, tag=tag, name=tag)
                    kc = KD // split
                    for k in range(0, KD, kc):
                        nc.sync.dma_start(
                            wt[:, k * JC:(k + kc) * JC],
                            dram[:, k * JC:(k + kc) * JC])
                    return wt

                def proj_qk(wt, xte, e, dst):
                    # dst[j][:, eslice] = rope((w x)^T)
                    for j in range(HPC):
                        acc = psA.tile([128, TE], F32, tag="qk")
                        for k in range(KD):
                            nc.tensor.matmul(
                                acc[:],
                                wt[:, k * JC + j * 128:k * JC + (j + 1) * 128],
                                xte[:, k * TE:(k + 1) * TE],
                                start=(k == 0), stop=(k == KD - 1),
                            )
                        rope(acc, dst[j], e * TE, TE, p_rt)

                def proj_v(xte, e):
                    for tl in range(TE // 128):
                        tt = e * (TE // 128) + tl
                        acc = psA.tile([128, JC], F32, tag="v", name="acc")
                        for k in range(KD):
                            nc.tensor.matmul(
                                acc[:],
                                xte[:, k * TE + tl * 128:
                                    k * TE + (tl + 1) * 128],
                                wv_t[:, k * JC:(k + 1) * JC],
                                start=(k == 0), stop=(k == KD - 1),
                            )
                        nc.vector.tensor_copy(V[tt][:], acc[:])

                # A1: K and V (wk, wv resident). Emission order matters:
                # the DMA pipe drains roughly in order, so interleave the wk
                # chunks with the first x quarter (K(q0) consumes both in k
                # order), then cs (first rope needs it ~13us in), then wv
                # (first V proj ~17us in), then the remaining x quarters.
                with tc.tile_pool(name="wkv", bufs=1) as p_w:
                    xtiles = [None] * N_TE
                    wk_t = p_w.tile([128, KD * JC], BF16, tag="wk", name="wk")
                    xte0 = p_x.tile([128, KD * TE], BF16, tag="xte0",
                                    name="xte0")
                    xtiles[0] = xte0
                    kc = KD // 4
                    for k4 in range(0, KD, kc):
                        nc.sync.dma_start(
                            wk_t[:, k4 * JC:(k4 + kc) * JC],
                            wkS[:, k4 * JC:(k4 + kc) * JC])
                        nc.sync.dma_start(
                            xte0[:, k4 * TE:(k4 + kc) * TE],
                            xS[:, k4 * TE:(k4 + kc) * TE])
                    nc.sync.dma_start(csa_t[:], csA[:])
                    nc.sync.dma_start(csb_t[:], csB[:])
                    wv_t = load_w(p_w, wvS, "wv", 4)
                    xtiles[1] = load_xte(1)
                    xtiles[2] = load_xte(2)
                    xtiles[3] = load_xte(3)
                    # wq shares the pool with wk/wv (no address reuse, so its
                    # DMA streams in during A1 instead of stalling A2)
                    wq_t = load_w(p_w, wqS, "wq", 4) if PHASES != "a1" else None

                    for e in range(N_TE):
                        proj_qk(wk_t, xtiles[e], e, KT)
                        proj_v(xtiles[e], e)

                    # A2: Q (wq resident) on the held x tiles
                    if PHASES != "a1":
                        for e in range(N_TE):
                            proj_qk(wq_t, xtiles[e], e, QT)

        # ---- phases B + C ----
        if PHASES not in ("a1", "a2"):
            with tc.tile_pool(name="wo", bufs=1) as p_wo, \
                 tc.tile_pool(name="pt", bufs=PT_BUFS) as p_pt, \
                 tc.tile_pool(name="ao", bufs=6) as p_ao, \
                 tc.tile_pool(name="bmisc", bufs=2) as p_bm, \
                 tc.tile_pool(name="pyout", bufs=int(_os.environ.get("K_PYO", "4"))) as p_po, \
                 tc.tile_pool(name="psSC", bufs=SC_BUFS, space="PSUM") as psSC, \
                 tc.tile_pool(name="psOU", bufs=int(_os.environ.get("K_OU_BUFS", "2")), space="PSUM") as psOU, \
                 tc.tile_pool(name="psSM", bufs=1, space="PSUM") as psSM, \
                 tc.tile_pool(name="psBC", bufs=1, space="PSUM") as psBC, \
                 tc.tile_pool(name="psC", bufs=int(_os.environ.get("K_PY_BUFS", "2")), space="PSUM") as psC:

                wo_t = p_wo.tile([128, HPC * D], BF16, tag="wo")
                nc.sync.dma_start(wo_t[:], woS[:])
                o1_t = p_bm.tile([128, 1], BF16, tag="o1")
                o2_t = p_bm.tile([1, 128], F32R, tag="o2")
                nc.sync.dma_start(o1_t[:], ones1[:])
                nc.sync.dma_start(o2_t[:], ones2[:])

                for qc in range(N_QC):
                    qs = qc * QC
                    ao = []
                    for h in range(HPC):
                        pt = p_pt.tile([128, KT_TILES * QC], BF16, tag="pt")
                        for k in range(KT_TILES):
                            sc = psSC.tile([128, QC], F32, tag="sc")
                            nc.tensor.matmul(
                                sc[:],
                                KT[h][:, k * 128:(k + 1) * 128],
                                QT[h][:, qs:qs + QC],
                                start=True, stop=True,
                            )
                            nc.scalar.activation(
                                pt[:, k * QC:(k + 1) * QC], sc[:], AF.Exp)
                        ou = psOU.tile([128, QC], F32, tag="ou")
                        sm = psSM.tile([1, QC], F32, tag="sm")
                        for k in range(KT_TILES):
                            nc.tensor.matmul(
                                ou[:],
                                V[k][:, h * 128:(h + 1) * 128],
                                pt[:, k * QC:(k + 1) * QC],
                                start=(k == 0), stop=(k == KT_TILES - 1),
                            )
                            nc.tensor.matmul(
                                sm[:], o1_t[:], pt[:, k * QC:(k + 1) * QC],
                                start=(k == 0), stop=(k == KT_TILES - 1),
                            )
                        rc = p_bm.tile([1, QC], F32R, tag="rc")
                        with nc.allow_low_precision(reason="softmax denom in tf32"):
                            nc.vector.reciprocal(rc[:], sm[:])
                        bc = psBC.tile([128, QC], F32, tag="bc")
                        nc.tensor.matmul(bc[:], o2_t[:], rc[:],
                                         start=True, stop=True)
                        # TT cannot read two PSUM operands; stage bc in SBUF
                        bc_sb = p_bm.tile([128, QC], F32, tag="bcsb")
                        nc.vector.tensor_copy(bc_sb[:], bc[:])
                        ao_h = p_ao.tile([128, QC], BF16, tag="ao")
                        nc.vector.tensor_tensor(ao_h[:], ou[:], bc_sb[:], ALU.mult)
                        ao.append(ao_h)

                    # phase C for this q-chunk
                    if PHASES == "b":
                        continue
                    for tl in range(QC // 128):
                        ts = qs + tl * 128
                        out_sb = p_po.tile([128, D], BF16, tag="pyo")
                        for ec in range(D // 512):
                            acc = psC.tile([128, 512], F32, tag="py")
                            for j in range(HPC):
                                nc.tensor.matmul(
                                    acc[:],
                                    ao[j][:, tl * 128:(tl + 1) * 128],
                                    wo_t[:, j * D + ec * 512:j * D + (ec + 1) * 512],
                                    start=(j == 0), stop=(j == HPC - 1),
                                )
                            nc.vector.tensor_copy(
                                out_sb[:, ec * 512:(ec + 1) * 512], acc[:])
                        nc.sync.dma_start(py[ts:ts + 128, :], out_sb[:])

    nc.compile()
    return nc


def _prep_inputs(x, freqs_cis, wq, wk, wv, wo):
    """Host-side shard + layout prep. Returns in_maps for the 8 cores."""
    import ml_dtypes

    BF = ml_dtypes.bfloat16
    scale = HD ** (-0.5)
    # even/odd permutation within each head's 128 rows
    perm = np.concatenate([np.arange(0, HD, 2), np.arange(1, HD, 2)])

    cos = np.ascontiguousarray(freqs_cis[:, :, 0].T, dtype=np.float32)  # (64,T)
    sin = np.ascontiguousarray(freqs_cis[:, :, 1].T, dtype=np.float32)
    csA = np.concatenate([cos, cos], axis=0)          # (128, T)
    csB = np.concatenate([-sin, sin], axis=0)         # (128, T)
    ones1 = np.ones((128, 1), BF)
    ones2 = np.ones((1, 128), np.float32)

    def tile_w(wT):
        # (D, JC) -> SBUF layout [128, KD*JC]: row p, col k*JC+j = wT[k*128+p, j]
        return np.ascontiguousarray(
            wT.reshape(KD, 128, JC).transpose(1, 0, 2).reshape(128, KD * JC)
        ).astype(BF)

    in_maps = []
    for c in range(N_CORES):
        b, g = divmod(c, 4)
        rows = slice(g * JC, (g + 1) * JC)
        wq_g = wq[rows].reshape(HPC, HD, D)[:, perm].reshape(JC, D) * scale
        wk_g = wk[rows].reshape(HPC, HD, D)[:, perm].reshape(JC, D)
        wv_g = wv[rows]
        wo_g = wo[:, rows]
        # x: [128, N_TE*KD*TE]: row p, col (e*KD+k)*TE+t = x[b][e*TE+t, k*128+p]
        xS = np.ascontiguousarray(
            x[b].reshape(N_TE, TE, KD, 128).transpose(3, 0, 2, 1)
            .reshape(128, N_TE * KD * TE)).astype(BF)
        # wo: [128, HPC*D]: row p, col j*D+e = wo_g.T[j*128+p, e]
        woS = np.ascontiguousarray(
            np.ascontiguousarray(wo_g.T).reshape(HPC, 128, D)
            .transpose(1, 0, 2).reshape(128, HPC * D)).astype(BF)
        in_maps.append({
            "xS": xS,
            "wqS": tile_w(np.ascontiguousarray(wq_g.T)),
            "wkS": tile_w(np.ascontiguousarray(wk_g.T)),
            "wvS": tile_w(np.ascontiguousarray(wv_g.T)),
            "woS": woS,
            "csA": csA,
            "csB": csB,
            "ones1": ones1,
            "ones2": ones2,
        })
    return in_maps


def _make_runner(nc):
    """Cacheable jitted SPMD runner (mirrors bass2jax.run_bass_via_pjrt's
    multi-core path, minus donation, so one jit serves repeated calls)."""
    import jax
    from concourse import mybir
    from concourse.bass2jax import (
        _bass_exec_p, install_neuronx_cc_hook, partition_id_tensor)
    from jax.experimental.shard_map import shard_map
    from jax.sharding import Mesh, NamedSharding, PartitionSpec

    install_neuronx_cc_hook()
    partition_name = (
        nc.partition_id_tensor.name if nc.partition_id_tensor else None)
    in_names, out_names, out_avals, zero_outs = [], [], [], []
    for alloc in nc.m.functions[0].allocations:
        if not isinstance(alloc, mybir.MemoryLocationSet):
            continue
        name = alloc.memorylocations[0].name
        if alloc.kind == "ExternalInput":
            if name != partition_name:
                in_names.append(name)
        elif alloc.kind == "ExternalOutput":
            out_names.append(name)
            shape = tuple(alloc.tensor_shape)
            dtype = mybir.dt.np(alloc.dtype)
            out_avals.append(jax.core.ShapedArray(shape, dtype))
            zero_outs.append(np.zeros(shape, dtype))
    all_in_names = list(in_names) + out_names
    if partition_name is not None:
        all_in_names.append(partition_name)

    def _body(*args):
        operands = list(args)
        if partition_name is not None:
            operands.append(partition_id_tensor())
        outs = _bass_exec_p.bind(
            *operands,
            out_avals=tuple(out_avals),
            in_names=tuple(all_in_names),
            out_names=tuple(out_names),
            lowering_input_output_aliases=(),
            sim_require_finite=True,
            sim_require_nnan=True,
            nc=nc,
        )
        return tuple(outs)

    devices = jax.devices()[:N_CORES]
    assert len(devices) == N_CORES, f"need {N_CORES} devices, got {devices}"
    mesh = Mesh(np.asarray(devices), ("core",))
    nshard = NamedSharding(mesh, PartitionSpec("core"))
    n_in = len(in_names) + len(out_names)
    jf = jax.jit(
        shard_map(_body, mesh=mesh,
                  in_specs=(PartitionSpec("core"),) * n_in,
                  out_specs=(PartitionSpec("core"),) * len(out_names),
                  check_rep=False),
        keep_unused=True,
    )
    dev_zero = [
        jax.device_put(
            np.zeros((N_CORES * z.shape[0], *z.shape[1:]), z.dtype), nshard)
        for z in zero_outs
    ]

    def run(in_maps):
        concat_in = [
            np.concatenate([np.asarray(in_maps[c][nm])
                            for c in range(N_CORES)], axis=0)
            for nm in in_names
        ]
        dev_in = [jax.device_put(a, nshard) for a in concat_in]
        outs = jf(*dev_in, *dev_zero)
        return {
            name: np.asarray(outs[i]) for i, name in enumerate(out_names)
        }

    return run


def kernel(x, freqs_cis, wq, wk, wv, wo):
    if "nc" not in _cache:
        _cache["nc"] = _build_program()
    if "run" not in _cache:
        _cache["run"] = _make_runner(_cache["nc"])

    in_maps = _prep_inputs(
        np.asarray(x), np.asarray(freqs_cis), np.asarray(wq),
        np.asarray(wk), np.asarray(wv), np.asarray(wo))
    outs = _cache["run"](in_maps)
    pys = outs["py"].astype(np.float32).reshape(N_CORES, T, D)

    out = np.empty((B, T, D), dtype=np.float32)
    for b in range(B):
        acc = pys[b * 4]
        for g in range(1, 4):
            acc = acc + pys[b * 4 + g]
        out[b] = acc
    return out


# revision 28
# speedup vs baseline: 1.3351x; 1.3202x over previous
"""Multi-head attention (B=2, T=2048, D=2048, H=16, HD=128) on 8 Trainium2
NeuronCores.

Sharding: core c in 0..7 handles batch b = c // 4 and head group g = c % 4
(4 heads per core) — tensor-parallel over heads within each batch element.
wq/wk/wv are column-sharded (rows of the (D,D) weight, since y = x @ W.T),
wo is row-sharded; the partial outputs (one per head group) are summed on
the host (the "all-reduce"), then the two batch elements are stacked.

Device kernel (per core, SPMD):
  phase A1: KT (roped) and V projections, streaming xT in t-quarters that
            stay resident in SBUF (bf16)
  phase A2: QT (roped, pre-scaled) projection from the held x tiles (no
            second x DMA)
  phase B:  per (q-chunk, head): scoresT = KT_k-tile.T @ QT (k on partitions,
            q on free dim), exp on ACT (no max subtraction — scores are
            O(5) so exp is safe in fp32), unnormalized out accumulated as
            V.T-matmul with exp(scores) as the moving operand (no PE
            transposes anywhere), softmax denominators via ones-matmul,
            normalization via a K=1 broadcast matmul + DVE multiply
  phase C:  per q-chunk: partial_y = aoT.T @ woT accumulated over the 4
            head k-steps, DMA'd out per (t-tile, e-chunk)

All matmul operands are bfloat16 (1 cycle/row on the PE like fp32r, but
half the DMA/SBUF footprint and fast-weight-load eligible); accumulation
stays fp32 in PSUM.  RoPE pairs are made partition-contiguous by permuting
the wq/wk output rows per head on the host (even hd components land in
partitions 0..63, odd in 64..127), which turns the rotation into four
full-width DVE ops against host-precomputed [cos;cos] and [-sin;sin]
tables. The softmax scale is folded into wq. The partial output py is
returned in bf16 and summed across head-group cores in fp32 on the host.
"""
from contextlib import ExitStack

import numpy as np

B, T, D, H = 2, 2048, 2048, 16
HD = D // H            # 128
N_CORES = 8
HPC = H // 4           # 4 heads per core
JC = HPC * HD          # 512 per-core projection width
KT_TILES = T // 128    # 16 k tiles
QC = 512               # q-chunk width in phase B
N_QC = T // QC         # 4
TE = 512               # t-quarter width in phase A
N_TE = T // TE         # 4
KD = D // 128          # 16 contraction tiles for the projections

import os as _os

SC_BUFS = int(_os.environ.get("K_SC_BUFS", "2"))
# analysis aid: truncate the program after phase a1 / a2 / b (default: full)
PHASES = _os.environ.get("K_PHASES", "full")
PSA_BUFS = int(_os.environ.get("K_PSA_BUFS", "4"))
RT_BUFS = int(_os.environ.get("K_RT_BUFS", "3"))
PT_BUFS = int(_os.environ.get("K_PT_BUFS", "2"))
XSPLIT = int(_os.environ.get("K_XSPLIT", "4"))    # xte DMA chunks

_cache = {}


def _build_program():
    import concourse.bacc as bacc
    import concourse.tile as tile
    from concourse import mybir

    F32 = mybir.dt.float32
    F32R = mybir.dt.float32r
    BF16 = mybir.dt.bfloat16
    AF = mybir.ActivationFunctionType
    ALU = mybir.AluOpType

    nc = bacc.Bacc("TRN2", target_bir_lowering=False, debug=False,
                   num_devices=N_CORES)

    # All inputs are pre-tiled on the host into the exact SBUF layout
    # ([128, free]) so every DMA is a dense contiguous copy with 2KB+ lines,
    # and packed into just TWO dram tensors: per-call dispatch cost of the
    # sharded jax call scales with the argument count (~40us/arg measured),
    # so 10 args -> 3 is worth ~300us/call.
    #   blobB (bf16): xS | wqS | wkS | wvS | woS | ones1-column
    #   csF  (fp32):  csA | csB | ones-row-block (row 0 read as [1,128])
    XO = 0
    WQO = XO + N_TE * KD * TE
    WKO = WQO + KD * JC
    WVO = WKO + KD * JC
    WOO = WVO + KD * JC
    O1O = WOO + HPC * D
    BW = O1O + 2
    blobB = nc.dram_tensor("blobB", [128, BW], BF16,
                           kind="ExternalInput").ap()
    csF = nc.dram_tensor("csF", [128, 2 * T + 128], F32,
                         kind="ExternalInput").ap()
    py = nc.dram_tensor("py", [T, D], BF16, kind="ExternalOutput").ap()

    with tile.TileContext(nc) as tc, ExitStack() as ctx:
        # long-lived pools on the RIGHT side of the SBUF heap (the tile
        # allocator is a per-side LIFO stack; phase-scoped pools live on the
        # default left side and can come and go underneath these)
        p_qkv = ctx.enter_context(tc.tile_pool(name="qkv", bufs=1,
                                               side="right"))

        KT = [p_qkv.tile([128, T], BF16, tag=f"KT{h}", name=f"KT{h}")
              for h in range(HPC)]
        V = [p_qkv.tile([128, JC], BF16, tag=f"V{t}", name=f"V{t}")
             for t in range(KT_TILES)]
        QT = [p_qkv.tile([128, T], BF16, tag=f"QT{h}", name=f"QT{h}")
              for h in range(HPC)]

        def rope(ps_tile, dst, t0, tw, pool_tmp):
            """dst[:, t0:t0+tw] = rotate(ps_tile) using csA/csB tables."""
            u = pool_tmp.tile([128, tw], F32, tag="ropeu")
            v = pool_tmp.tile([128, tw], F32, tag="ropev")
            nc.vector.tensor_tensor(u[:], ps_tile[:], csa_t[:, t0:t0 + tw],
                                    ALU.mult)
            nc.vector.tensor_tensor(v[0:64, :], ps_tile[64:128, :],
                                    csb_t[0:64, t0:t0 + tw], ALU.mult)
            nc.vector.tensor_tensor(v[64:128, :], ps_tile[0:64, :],
                                    csb_t[64:128, t0:t0 + tw], ALU.mult)
            nc.vector.tensor_tensor(dst[:, t0:t0 + tw], u[:], v[:], ALU.add)

        # ---- phase A: projections ----
        # cs pool is ctx-scoped: its trailing ones-row serves phase B's
        # broadcast matmul
        p_cs = ctx.enter_context(tc.tile_pool(name="cs", bufs=1))
        cs_t = p_cs.tile([128, 2 * T + 128], F32, tag="cs")
        csa_t = cs_t[:, 0:T]
        csb_t = cs_t[:, T:2 * T]
        if True:
            with tc.tile_pool(name="xa", bufs=1) as p_x, \
                 tc.tile_pool(name="ropetmp", bufs=RT_BUFS) as p_rt, \
                 tc.tile_pool(name="psA", bufs=PSA_BUFS, space="PSUM") as psA:

                def load_xte(e, split=1):
                    # split per k-group so the first matmul of the quarter
                    # waits on a fraction of the 2MB, not all of it
                    xte = p_x.tile([128, KD * TE], BF16, tag=f"xte{e}",
                                   name=f"xte{e}")
                    kc = KD // split
                    base = XO + e * KD * TE
                    for k4 in range(0, KD, kc):
                        nc.sync.dma_start(
                            xte[:, k4 * TE:(k4 + kc) * TE],
                            blobB[:, base + k4 * TE:base + (k4 + kc) * TE])
                    return xte

                def load_w(pool, off, tag, split):
                    # per-k-chunk DMAs: first projection matmul only waits
                    # for its own k slice instead of the full weight
                    wt = pool.tile([128, KD * JC], BF16, tag=tag, name=tag)
                    kc = KD // split
                    for k in range(0, KD, kc):
                        nc.sync.dma_start(
                            wt[:, k * JC:(k + kc) * JC],
                            blobB[:, off + k * JC:off + (k + kc) * JC])
                    return wt

                def proj_qk(wt, xte, e, dst):
                    # dst[j][:, eslice] = rope((w x)^T)
                    for j in range(HPC):
                        acc = psA.tile([128, TE], F32, tag="qk")
                        for k in range(KD):
                            nc.tensor.matmul(
                                acc[:],
                                wt[:, k * JC + j * 128:k * JC + (j + 1) * 128],
                                xte[:, k * TE:(k + 1) * TE],
                                start=(k == 0), stop=(k == KD - 1),
                            )
                        rope(acc, dst[j], e * TE, TE, p_rt)

                def proj_v(xte, e):
                    for tl in range(TE // 128):
                        tt = e * (TE // 128) + tl
                        acc = psA.tile([128, JC], F32, tag="v", name="acc")
                        for k in range(KD):
                            nc.tensor.matmul(
                                acc[:],
                                xte[:, k * TE + tl * 128:
                                    k * TE + (tl + 1) * 128],
                                wv_t[:, k * JC:(k + 1) * JC],
                                start=(k == 0), stop=(k == KD - 1),
                            )
                        nc.vector.tensor_copy(V[tt][:], acc[:])

                # A1: K and V (wk, wv resident). Emission order matters:
                # the DMA pipe drains roughly in order, so interleave the wk
                # chunks with the first x quarter (K(q0) consumes both in k
                # order), then cs (first rope needs it ~13us in), then wv
                # (first V proj ~17us in), then the remaining x quarters.
                with tc.tile_pool(name="wkv", bufs=1) as p_w:
                    xtiles = [None] * N_TE
                    wk_t = p_w.tile([128, KD * JC], BF16, tag="wk", name="wk")
                    xte0 = p_x.tile([128, KD * TE], BF16, tag="xte0",
                                    name="xte0")
                    xtiles[0] = xte0
                    kc = KD // 4
                    for k4 in range(0, KD, kc):
                        nc.sync.dma_start(
                            wk_t[:, k4 * JC:(k4 + kc) * JC],
                            blobB[:, WKO + k4 * JC:WKO + (k4 + kc) * JC])
                        nc.sync.dma_start(
                            xte0[:, k4 * TE:(k4 + kc) * TE],
                            blobB[:, XO + k4 * TE:XO + (k4 + kc) * TE])
                    nc.sync.dma_start(cs_t[:], csF[:])
                    wv_t = load_w(p_w, WVO, "wv", 4)
                    xtiles[1] = load_xte(1)
                    xtiles[2] = load_xte(2)
                    xtiles[3] = load_xte(3)
                    # wq shares the pool with wk/wv (no address reuse, so its
                    # DMA streams in during A1 instead of stalling A2)
                    wq_t = load_w(p_w, WQO, "wq", 4) if PHASES != "a1" else None

                    for e in range(N_TE):
                        proj_qk(wk_t, xtiles[e], e, KT)
                        proj_v(xtiles[e], e)

                    # A2: Q (wq resident) on the held x tiles
                    if PHASES != "a1":
                        for e in range(N_TE):
                            proj_qk(wq_t, xtiles[e], e, QT)

        # ---- phases B + C ----
        if PHASES not in ("a1", "a2"):
            with tc.tile_pool(name="wo", bufs=1) as p_wo, \
                 tc.tile_pool(name="pt", bufs=PT_BUFS) as p_pt, \
                 tc.tile_pool(name="ao", bufs=6) as p_ao, \
                 tc.tile_pool(name="bmisc", bufs=2) as p_bm, \
                 tc.tile_pool(name="pyout", bufs=int(_os.environ.get("K_PYO", "4"))) as p_po, \
                 tc.tile_pool(name="psSC", bufs=SC_BUFS, space="PSUM") as psSC, \
                 tc.tile_pool(name="psOU", bufs=int(_os.environ.get("K_OU_BUFS", "2")), space="PSUM") as psOU, \
                 tc.tile_pool(name="psSM", bufs=1, space="PSUM") as psSM, \
                 tc.tile_pool(name="psBC", bufs=1, space="PSUM") as psBC, \
                 tc.tile_pool(name="psC", bufs=int(_os.environ.get("K_PY_BUFS", "2")), space="PSUM") as psC:

                wo_t = p_wo.tile([128, HPC * D], BF16, tag="wo")
                nc.sync.dma_start(wo_t[:], blobB[:, WOO:WOO + HPC * D])
                o1_t = p_bm.tile([128, 1], BF16, tag="o1")
                nc.sync.dma_start(o1_t[:], blobB[:, O1O:O1O + 1])
                o2_t = p_bm.tile([1, 128], F32R, tag="o2")
                nc.vector.tensor_copy(o2_t[:], cs_t[0:1, 2 * T:2 * T + 128])

                for qc in range(N_QC):
                    qs = qc * QC
                    ao = []
                    for h in range(HPC):
                        pt = p_pt.tile([128, KT_TILES * QC], BF16, tag="pt")
                        for k in range(KT_TILES):
                            sc = psSC.tile([128, QC], F32, tag="sc")
                            nc.tensor.matmul(
                                sc[:],
                                KT[h][:, k * 128:(k + 1) * 128],
                                QT[h][:, qs:qs + QC],
                                start=True, stop=True,
                            )
                            nc.scalar.activation(
                                pt[:, k * QC:(k + 1) * QC], sc[:], AF.Exp)
                        ou = psOU.tile([128, QC], F32, tag="ou")
                        sm = psSM.tile([1, QC], F32, tag="sm")
                        for k in range(KT_TILES):
                            nc.tensor.matmul(
                                ou[:],
                                V[k][:, h * 128:(h + 1) * 128],
                                pt[:, k * QC:(k + 1) * QC],
                                start=(k == 0), stop=(k == KT_TILES - 1),
                            )
                            nc.tensor.matmul(
                                sm[:], o1_t[:], pt[:, k * QC:(k + 1) * QC],
                                start=(k == 0), stop=(k == KT_TILES - 1),
                            )
                        rc = p_bm.tile([1, QC], F32R, tag="rc")
                        with nc.allow_low_precision(reason="softmax denom in tf32"):
                            nc.vector.reciprocal(rc[:], sm[:])
                        bc = psBC.tile([128, QC], F32, tag="bc")
                        nc.tensor.matmul(bc[:], o2_t[:], rc[:],
                                         start=True, stop=True)
                        # TT cannot read two PSUM operands; stage bc in SBUF
                        bc_sb = p_bm.tile([128, QC], F32, tag="bcsb")
                        nc.vector.tensor_copy(bc_sb[:], bc[:])
                        ao_h = p_ao.tile([128, QC], BF16, tag="ao")
                        nc.vector.tensor_tensor(ao_h[:], ou[:], bc_sb[:], ALU.mult)
                        ao.append(ao_h)

                    # phase C for this q-chunk
                    if PHASES == "b":
                        continue
                    for tl in range(QC // 128):
                        ts = qs + tl * 128
                        out_sb = p_po.tile([128, D], BF16, tag="pyo")
                        for ec in range(D // 512):
                            acc = psC.tile([128, 512], F32, tag="py")
                            for j in range(HPC):
                                nc.tensor.matmul(
                                    acc[:],
                                    ao[j][:, tl * 128:(tl + 1) * 128],
                                    wo_t[:, j * D + ec * 512:j * D + (ec + 1) * 512],
                                    start=(j == 0), stop=(j == HPC - 1),
                                )
                            nc.vector.tensor_copy(
                                out_sb[:, ec * 512:(ec + 1) * 512], acc[:])
                        nc.sync.dma_start(py[ts:ts + 128, :], out_sb[:])

    nc.compile()
    return nc


def _prep_inputs(x, freqs_cis, wq, wk, wv, wo):
    """Host-side shard + layout prep. Returns in_maps for the 8 cores."""
    import ml_dtypes

    BF = ml_dtypes.bfloat16
    scale = HD ** (-0.5)
    # even/odd permutation within each head's 128 rows
    perm = np.concatenate([np.arange(0, HD, 2), np.arange(1, HD, 2)])

    cos = np.ascontiguousarray(freqs_cis[:, :, 0].T, dtype=np.float32)  # (64,T)
    sin = np.ascontiguousarray(freqs_cis[:, :, 1].T, dtype=np.float32)
    csA = np.concatenate([cos, cos], axis=0)          # (128, T)
    csB = np.concatenate([-sin, sin], axis=0)         # (128, T)
    # csF = csA | csB | ones block (row 0 of the last 128 cols = [1,128] ones)
    csF = np.concatenate([csA, csB, np.ones((128, 128), np.float32)], axis=1)

    def tile_w(wT):
        # (D, JC) -> SBUF layout [128, KD*JC]: row p, col k*JC+j = wT[k*128+p, j]
        return np.ascontiguousarray(
            wT.reshape(KD, 128, JC).transpose(1, 0, 2).reshape(128, KD * JC)
        ).astype(BF)

    in_maps = []
    for c in range(N_CORES):
        b, g = divmod(c, 4)
        rows = slice(g * JC, (g + 1) * JC)
        wq_g = wq[rows].reshape(HPC, HD, D)[:, perm].reshape(JC, D) * scale
        wk_g = wk[rows].reshape(HPC, HD, D)[:, perm].reshape(JC, D)
        wv_g = wv[rows]
        wo_g = wo[:, rows]
        # x: [128, N_TE*KD*TE]: row p, col (e*KD+k)*TE+t = x[b][e*TE+t, k*128+p]
        xS = np.ascontiguousarray(
            x[b].reshape(N_TE, TE, KD, 128).transpose(3, 0, 2, 1)
            .reshape(128, N_TE * KD * TE)).astype(BF)
        # wo: [128, HPC*D]: row p, col j*D+e = wo_g.T[j*128+p, e]
        woS = np.ascontiguousarray(
            np.ascontiguousarray(wo_g.T).reshape(HPC, 128, D)
            .transpose(1, 0, 2).reshape(128, HPC * D)).astype(BF)
        blobB = np.concatenate([
            xS,
            tile_w(np.ascontiguousarray(wq_g.T)),
            tile_w(np.ascontiguousarray(wk_g.T)),
            tile_w(np.ascontiguousarray(wv_g.T)),
            woS,
            np.ones((128, 2), BF),
        ], axis=1)
        in_maps.append({"blobB": blobB, "csF": csF})
    return in_maps


def _make_runner(nc):
    """Cacheable jitted SPMD runner (mirrors bass2jax.run_bass_via_pjrt's
    multi-core path, minus donation, so one jit serves repeated calls)."""
    import jax
    from concourse import mybir
    from concourse.bass2jax import (
        _bass_exec_p, install_neuronx_cc_hook, partition_id_tensor)
    from jax.experimental.shard_map import shard_map
    from jax.sharding import Mesh, NamedSharding, PartitionSpec

    install_neuronx_cc_hook()
    partition_name = (
        nc.partition_id_tensor.name if nc.partition_id_tensor else None)
    in_names, out_names, out_avals, zero_outs = [], [], [], []
    for alloc in nc.m.functions[0].allocations:
        if not isinstance(alloc, mybir.MemoryLocationSet):
            continue
        name = alloc.memorylocations[0].name
        if alloc.kind == "ExternalInput":
            if name != partition_name:
                in_names.append(name)
        elif alloc.kind == "ExternalOutput":
            out_names.append(name)
            shape = tuple(alloc.tensor_shape)
            dtype = mybir.dt.np(alloc.dtype)
            out_avals.append(jax.core.ShapedArray(shape, dtype))
            zero_outs.append(np.zeros(shape, dtype))
    all_in_names = list(in_names) + out_names
    if partition_name is not None:
        all_in_names.append(partition_name)

    def _body(*args):
        operands = list(args)
        if partition_name is not None:
            operands.append(partition_id_tensor())
        outs = _bass_exec_p.bind(
            *operands,
            out_avals=tuple(out_avals),
            in_names=tuple(all_in_names),
            out_names=tuple(out_names),
            lowering_input_output_aliases=(),
            sim_require_finite=True,
            sim_require_nnan=True,
            nc=nc,
        )
        return tuple(outs)

    devices = jax.devices()[:N_CORES]
    assert len(devices) == N_CORES, f"need {N_CORES} devices, got {devices}"
    mesh = Mesh(np.asarray(devices), ("core",))
    nshard = NamedSharding(mesh, PartitionSpec("core"))
    n_in = len(in_names) + len(out_names)
    jf = jax.jit(
        shard_map(_body, mesh=mesh,
                  in_specs=(PartitionSpec("core"),) * n_in,
                  out_specs=(PartitionSpec("core"),) * len(out_names),
                  check_rep=False),
        keep_unused=True,
    )
    dev_zero = [
        jax.device_put(
            np.zeros((N_CORES * z.shape[0], *z.shape[1:]), z.dtype), nshard)
        for z in zero_outs
    ]

    def run(in_maps):
        concat_in = [
            np.concatenate([np.asarray(in_maps[c][nm])
                            for c in range(N_CORES)], axis=0)
            for nm in in_names
        ]
        dev_in = [jax.device_put(a, nshard) for a in concat_in]
        outs = jf(*dev_in, *dev_zero)
        return {
            name: np.asarray(outs[i]) for i, name in enumerate(out_names)
        }

    return run


def kernel(x, freqs_cis, wq, wk, wv, wo):
    if "nc" not in _cache:
        _cache["nc"] = _build_program()
    if "run" not in _cache:
        _cache["run"] = _make_runner(_cache["nc"])

    in_maps = _prep_inputs(
        np.asarray(x), np.asarray(freqs_cis), np.asarray(wq),
        np.asarray(wk), np.asarray(wv), np.asarray(wo))
    outs = _cache["run"](in_maps)
    pys = outs["py"].astype(np.float32).reshape(N_CORES, T, D)

    out = np.empty((B, T, D), dtype=np.float32)
    for b in range(B):
        acc = pys[b * 4]
        for g in range(1, 4):
            acc = acc + pys[b * 4 + g]
        out[b] = acc
    return out


# revision 29
# speedup vs baseline: 2.5302x; 1.8951x over previous
"""Multi-head attention (B=2, T=2048, D=2048, H=16, HD=128) on 8 Trainium2
NeuronCores.

Sharding: core c in 0..7 handles batch b = c // 4 and head group g = c % 4
(4 heads per core) — tensor-parallel over heads within each batch element.
wq/wk/wv are column-sharded (rows of the (D,D) weight, since y = x @ W.T),
wo is row-sharded; the partial outputs (one per head group) are summed on
the host (the "all-reduce"), then the two batch elements are stacked.

Device kernel (per core, SPMD):
  phase A1: KT (roped) and V projections, streaming xT in t-quarters that
            stay resident in SBUF (bf16)
  phase A2: QT (roped, pre-scaled) projection from the held x tiles (no
            second x DMA)
  phase B:  per (q-chunk, head): scoresT = KT_k-tile.T @ QT (k on partitions,
            q on free dim), exp on ACT (no max subtraction — scores are
            O(5) so exp is safe in fp32), unnormalized out accumulated as
            V.T-matmul with exp(scores) as the moving operand (no PE
            transposes anywhere), softmax denominators via ones-matmul,
            normalization via a K=1 broadcast matmul + DVE multiply
  phase C:  per q-chunk: partial_y = aoT.T @ woT accumulated over the 4
            head k-steps, DMA'd out per (t-tile, e-chunk)

All matmul operands are bfloat16 (1 cycle/row on the PE like fp32r, but
half the DMA/SBUF footprint and fast-weight-load eligible); accumulation
stays fp32 in PSUM.  RoPE pairs are made partition-contiguous by permuting
the wq/wk output rows per head on the host (even hd components land in
partitions 0..63, odd in 64..127), which turns the rotation into four
full-width DVE ops against host-precomputed [cos;cos] and [-sin;sin]
tables. The softmax scale is folded into wq. The partial output py is
returned in bf16 and summed across head-group cores in fp32 on the host.
"""
from contextlib import ExitStack

import numpy as np

B, T, D, H = 2, 2048, 2048, 16
HD = D // H            # 128
N_CORES = 8
HPC = H // 4           # 4 heads per core
JC = HPC * HD          # 512 per-core projection width
KT_TILES = T // 128    # 16 k tiles
QC = 512               # q-chunk width in phase B
N_QC = T // QC         # 4
TE = 512               # t-quarter width in phase A
N_TE = T // TE         # 4
KD = D // 128          # 16 contraction tiles for the projections

import os as _os

SC_BUFS = int(_os.environ.get("K_SC_BUFS", "2"))
# analysis aid: truncate the program after phase a1 / a2 / b (default: full)
PHASES = _os.environ.get("K_PHASES", "full")
PSA_BUFS = int(_os.environ.get("K_PSA_BUFS", "4"))
RT_BUFS = int(_os.environ.get("K_RT_BUFS", "3"))
PT_BUFS = int(_os.environ.get("K_PT_BUFS", "2"))
XSPLIT = int(_os.environ.get("K_XSPLIT", "4"))    # xte DMA chunks

_cache = {}


def _build_program():
    import concourse.bacc as bacc
    import concourse.tile as tile
    from concourse import mybir

    F32 = mybir.dt.float32
    F32R = mybir.dt.float32r
    BF16 = mybir.dt.bfloat16
    AF = mybir.ActivationFunctionType
    ALU = mybir.AluOpType

    nc = bacc.Bacc("TRN2", target_bir_lowering=False, debug=False,
                   num_devices=N_CORES)

    # All inputs are pre-tiled on the host into the exact SBUF layout
    # ([128, free]) so every DMA is a dense contiguous copy with 2KB+ lines,
    # and packed into just TWO dram tensors: per-call dispatch cost of the
    # sharded jax call scales with the argument count (~40us/arg measured),
    # so 10 args -> 3 is worth ~300us/call.
    #   blobB (bf16): xS | wqS | wkS | wvS | woS | ones1-column
    #   csF  (fp32):  csA | csB | ones-row-block (row 0 read as [1,128])
    XO = 0
    WQO = XO + N_TE * KD * TE
    WKO = WQO + KD * JC
    WVO = WKO + KD * JC
    WOO = WVO + KD * JC
    O1O = WOO + HPC * D
    CSO = O1O + 2
    BW = CSO + 2 * T + 128
    blobB = nc.dram_tensor("blobB", [128, BW], BF16,
                           kind="ExternalInput").ap()
    py = nc.dram_tensor("py", [T, D], BF16, kind="ExternalOutput").ap()

    with tile.TileContext(nc) as tc, ExitStack() as ctx:
        # long-lived pools on the RIGHT side of the SBUF heap (the tile
        # allocator is a per-side LIFO stack; phase-scoped pools live on the
        # default left side and can come and go underneath these)
        p_qkv = ctx.enter_context(tc.tile_pool(name="qkv", bufs=1,
                                               side="right"))

        KT = [p_qkv.tile([128, T], BF16, tag=f"KT{h}", name=f"KT{h}")
              for h in range(HPC)]
        V = [p_qkv.tile([128, JC], BF16, tag=f"V{t}", name=f"V{t}")
             for t in range(KT_TILES)]
        QT = [p_qkv.tile([128, T], BF16, tag=f"QT{h}", name=f"QT{h}")
              for h in range(HPC)]

        def rope(ps_tile, dst, t0, tw, pool_tmp):
            """dst[:, t0:t0+tw] = rotate(ps_tile) using csA/csB tables."""
            u = pool_tmp.tile([128, tw], F32, tag="ropeu")
            v = pool_tmp.tile([128, tw], F32, tag="ropev")
            nc.vector.tensor_tensor(u[:], ps_tile[:], csa_t[:, t0:t0 + tw],
                                    ALU.mult)
            nc.vector.tensor_tensor(v[0:64, :], ps_tile[64:128, :],
                                    csb_t[0:64, t0:t0 + tw], ALU.mult)
            nc.vector.tensor_tensor(v[64:128, :], ps_tile[0:64, :],
                                    csb_t[64:128, t0:t0 + tw], ALU.mult)
            nc.vector.tensor_tensor(dst[:, t0:t0 + tw], u[:], v[:], ALU.add)

        # ---- phase A: projections ----
        # cs pool is ctx-scoped: its trailing ones-row serves phase B's
        # broadcast matmul
        p_cs = ctx.enter_context(tc.tile_pool(name="cs", bufs=1))
        cs_t = p_cs.tile([128, 2 * T + 128], BF16, tag="cs")
        csa_t = cs_t[:, 0:T]
        csb_t = cs_t[:, T:2 * T]
        if True:
            with tc.tile_pool(name="xa", bufs=1) as p_x, \
                 tc.tile_pool(name="ropetmp", bufs=RT_BUFS) as p_rt, \
                 tc.tile_pool(name="psA", bufs=PSA_BUFS, space="PSUM") as psA:

                def load_xte(e, split=1):
                    # split per k-group so the first matmul of the quarter
                    # waits on a fraction of the 2MB, not all of it
                    xte = p_x.tile([128, KD * TE], BF16, tag=f"xte{e}",
                                   name=f"xte{e}")
                    kc = KD // split
                    base = XO + e * KD * TE
                    for k4 in range(0, KD, kc):
                        nc.sync.dma_start(
                            xte[:, k4 * TE:(k4 + kc) * TE],
                            blobB[:, base + k4 * TE:base + (k4 + kc) * TE])
                    return xte

                def load_w(pool, off, tag, split):
                    # per-k-chunk DMAs: first projection matmul only waits
                    # for its own k slice instead of the full weight
                    wt = pool.tile([128, KD * JC], BF16, tag=tag, name=tag)
                    kc = KD // split
                    for k in range(0, KD, kc):
                        nc.sync.dma_start(
                            wt[:, k * JC:(k + kc) * JC],
                            blobB[:, off + k * JC:off + (k + kc) * JC])
                    return wt

                def proj_qk(wt, xte, e, dst):
                    # dst[j][:, eslice] = rope((w x)^T)
                    for j in range(HPC):
                        acc = psA.tile([128, TE], F32, tag="qk")
                        for k in range(KD):
                            nc.tensor.matmul(
                                acc[:],
                                wt[:, k * JC + j * 128:k * JC + (j + 1) * 128],
                                xte[:, k * TE:(k + 1) * TE],
                                start=(k == 0), stop=(k == KD - 1),
                            )
                        rope(acc, dst[j], e * TE, TE, p_rt)

                def proj_v(xte, e):
                    for tl in range(TE // 128):
                        tt = e * (TE // 128) + tl
                        acc = psA.tile([128, JC], F32, tag="v", name="acc")
                        for k in range(KD):
                            nc.tensor.matmul(
                                acc[:],
                                xte[:, k * TE + tl * 128:
                                    k * TE + (tl + 1) * 128],
                                wv_t[:, k * JC:(k + 1) * JC],
                                start=(k == 0), stop=(k == KD - 1),
                            )
                        nc.vector.tensor_copy(V[tt][:], acc[:])

                # A1: K and V (wk, wv resident). Emission order matters:
                # the DMA pipe drains roughly in order, so interleave the wk
                # chunks with the first x quarter (K(q0) consumes both in k
                # order), then cs (first rope needs it ~13us in), then wv
                # (first V proj ~17us in), then the remaining x quarters.
                with tc.tile_pool(name="wkv", bufs=1) as p_w:
                    xtiles = [None] * N_TE
                    wk_t = p_w.tile([128, KD * JC], BF16, tag="wk", name="wk")
                    xte0 = p_x.tile([128, KD * TE], BF16, tag="xte0",
                                    name="xte0")
                    xtiles[0] = xte0
                    kc = KD // 4
                    for k4 in range(0, KD, kc):
                        nc.sync.dma_start(
                            wk_t[:, k4 * JC:(k4 + kc) * JC],
                            blobB[:, WKO + k4 * JC:WKO + (k4 + kc) * JC])
                        nc.sync.dma_start(
                            xte0[:, k4 * TE:(k4 + kc) * TE],
                            blobB[:, XO + k4 * TE:XO + (k4 + kc) * TE])
                    nc.sync.dma_start(cs_t[:], blobB[:, CSO:CSO + 2 * T + 128])
                    wv_t = load_w(p_w, WVO, "wv", 4)
                    xtiles[1] = load_xte(1)
                    xtiles[2] = load_xte(2)
                    xtiles[3] = load_xte(3)
                    # wq shares the pool with wk/wv (no address reuse, so its
                    # DMA streams in during A1 instead of stalling A2)
                    wq_t = load_w(p_w, WQO, "wq", 4) if PHASES != "a1" else None

                    for e in range(N_TE):
                        proj_qk(wk_t, xtiles[e], e, KT)
                        proj_v(xtiles[e], e)

                    # A2: Q (wq resident) on the held x tiles
                    if PHASES != "a1":
                        for e in range(N_TE):
                            proj_qk(wq_t, xtiles[e], e, QT)

        # ---- phases B + C ----
        if PHASES not in ("a1", "a2"):
            with tc.tile_pool(name="wo", bufs=1) as p_wo, \
                 tc.tile_pool(name="pt", bufs=PT_BUFS) as p_pt, \
                 tc.tile_pool(name="ao", bufs=6) as p_ao, \
                 tc.tile_pool(name="bmisc", bufs=2) as p_bm, \
                 tc.tile_pool(name="pyout", bufs=int(_os.environ.get("K_PYO", "4"))) as p_po, \
                 tc.tile_pool(name="psSC", bufs=SC_BUFS, space="PSUM") as psSC, \
                 tc.tile_pool(name="psOU", bufs=int(_os.environ.get("K_OU_BUFS", "2")), space="PSUM") as psOU, \
                 tc.tile_pool(name="psSM", bufs=1, space="PSUM") as psSM, \
                 tc.tile_pool(name="psBC", bufs=1, space="PSUM") as psBC, \
                 tc.tile_pool(name="psC", bufs=int(_os.environ.get("K_PY_BUFS", "2")), space="PSUM") as psC:

                wo_t = p_wo.tile([128, HPC * D], BF16, tag="wo")
                nc.sync.dma_start(wo_t[:], blobB[:, WOO:WOO + HPC * D])
                o1_t = p_bm.tile([128, 1], BF16, tag="o1")
                nc.sync.dma_start(o1_t[:], blobB[:, O1O:O1O + 1])
                o2_t = p_bm.tile([1, 128], F32R, tag="o2")
                nc.vector.tensor_copy(o2_t[:], cs_t[0:1, 2 * T:2 * T + 128])

                for qc in range(N_QC):
                    qs = qc * QC
                    ao = []
                    for h in range(HPC):
                        pt = p_pt.tile([128, KT_TILES * QC], BF16, tag="pt")
                        for k in range(KT_TILES):
                            sc = psSC.tile([128, QC], F32, tag="sc")
                            nc.tensor.matmul(
                                sc[:],
                                KT[h][:, k * 128:(k + 1) * 128],
                                QT[h][:, qs:qs + QC],
                                start=True, stop=True,
                            )
                            nc.scalar.activation(
                                pt[:, k * QC:(k + 1) * QC], sc[:], AF.Exp)
                        ou = psOU.tile([128, QC], F32, tag="ou")
                        sm = psSM.tile([1, QC], F32, tag="sm")
                        for k in range(KT_TILES):
                            nc.tensor.matmul(
                                ou[:],
                                V[k][:, h * 128:(h + 1) * 128],
                                pt[:, k * QC:(k + 1) * QC],
                                start=(k == 0), stop=(k == KT_TILES - 1),
                            )
                            nc.tensor.matmul(
                                sm[:], o1_t[:], pt[:, k * QC:(k + 1) * QC],
                                start=(k == 0), stop=(k == KT_TILES - 1),
                            )
                        rc = p_bm.tile([1, QC], F32R, tag="rc")
                        with nc.allow_low_precision(reason="softmax denom in tf32"):
                            nc.vector.reciprocal(rc[:], sm[:])
                        bc = psBC.tile([128, QC], F32, tag="bc")
                        nc.tensor.matmul(bc[:], o2_t[:], rc[:],
                                         start=True, stop=True)
                        # TT cannot read two PSUM operands; stage bc in SBUF
                        bc_sb = p_bm.tile([128, QC], F32, tag="bcsb")
                        nc.vector.tensor_copy(bc_sb[:], bc[:])
                        ao_h = p_ao.tile([128, QC], BF16, tag="ao")
                        nc.vector.tensor_tensor(ao_h[:], ou[:], bc_sb[:], ALU.mult)
                        ao.append(ao_h)

                    # phase C for this q-chunk
                    if PHASES == "b":
                        continue
                    for tl in range(QC // 128):
                        ts = qs + tl * 128
                        out_sb = p_po.tile([128, D], BF16, tag="pyo")
                        for ec in range(D // 512):
                            acc = psC.tile([128, 512], F32, tag="py")
                            for j in range(HPC):
                                nc.tensor.matmul(
                                    acc[:],
                                    ao[j][:, tl * 128:(tl + 1) * 128],
                                    wo_t[:, j * D + ec * 512:j * D + (ec + 1) * 512],
                                    start=(j == 0), stop=(j == HPC - 1),
                                )
                            nc.vector.tensor_copy(
                                out_sb[:, ec * 512:(ec + 1) * 512], acc[:])
                        nc.sync.dma_start(py[ts:ts + 128, :], out_sb[:])

    nc.compile()
    return nc


def _prep_inputs(x, freqs_cis, wq, wk, wv, wo):
    """Host-side shard + layout prep. Returns in_maps for the 8 cores."""
    import ml_dtypes

    BF = ml_dtypes.bfloat16
    scale = HD ** (-0.5)
    # even/odd permutation within each head's 128 rows
    perm = np.concatenate([np.arange(0, HD, 2), np.arange(1, HD, 2)])

    cos = np.ascontiguousarray(freqs_cis[:, :, 0].T, dtype=np.float32)  # (64,T)
    sin = np.ascontiguousarray(freqs_cis[:, :, 1].T, dtype=np.float32)
    csA = np.concatenate([cos, cos], axis=0)          # (128, T)
    csB = np.concatenate([-sin, sin], axis=0)         # (128, T)
    # cs block (bf16): csA | csB | ones block (row 0 read as [1,128] ones)
    csBlk = np.concatenate([csA, csB, np.ones((128, 128), np.float32)],
                           axis=1).astype(BF)

    def tile_w(wT):
        # (D, JC) -> SBUF layout [128, KD*JC]: row p, col k*JC+j = wT[k*128+p, j]
        return np.ascontiguousarray(
            wT.reshape(KD, 128, JC).transpose(1, 0, 2).reshape(128, KD * JC)
        ).astype(BF)

    in_maps = []
    for c in range(N_CORES):
        b, g = divmod(c, 4)
        rows = slice(g * JC, (g + 1) * JC)
        wq_g = wq[rows].reshape(HPC, HD, D)[:, perm].reshape(JC, D) * scale
        wk_g = wk[rows].reshape(HPC, HD, D)[:, perm].reshape(JC, D)
        wv_g = wv[rows]
        wo_g = wo[:, rows]
        # x: [128, N_TE*KD*TE]: row p, col (e*KD+k)*TE+t = x[b][e*TE+t, k*128+p]
        xS = np.ascontiguousarray(
            x[b].reshape(N_TE, TE, KD, 128).transpose(3, 0, 2, 1)
            .reshape(128, N_TE * KD * TE)).astype(BF)
        # wo: [128, HPC*D]: row p, col j*D+e = wo_g.T[j*128+p, e]
        woS = np.ascontiguousarray(
            np.ascontiguousarray(wo_g.T).reshape(HPC, 128, D)
            .transpose(1, 0, 2).reshape(128, HPC * D)).astype(BF)
        blobB = np.concatenate([
            xS,
            tile_w(np.ascontiguousarray(wq_g.T)),
            tile_w(np.ascontiguousarray(wk_g.T)),
            tile_w(np.ascontiguousarray(wv_g.T)),
            woS,
            np.ones((128, 2), BF),
            csBlk,
        ], axis=1)
        in_maps.append({"blobB": blobB})
    return in_maps


def _make_runner(nc):
    """Cacheable jitted SPMD runner (mirrors bass2jax.run_bass_via_pjrt's
    multi-core path, minus donation, so one jit serves repeated calls)."""
    import jax
    from concourse import mybir
    from concourse.bass2jax import (
        _bass_exec_p, install_neuronx_cc_hook, partition_id_tensor)
    from jax.experimental.shard_map import shard_map
    from jax.sharding import Mesh, NamedSharding, PartitionSpec

    install_neuronx_cc_hook()
    partition_name = (
        nc.partition_id_tensor.name if nc.partition_id_tensor else None)
    in_names, out_names, out_avals, zero_outs = [], [], [], []
    for alloc in nc.m.functions[0].allocations:
        if not isinstance(alloc, mybir.MemoryLocationSet):
            continue
        name = alloc.memorylocations[0].name
        if alloc.kind == "ExternalInput":
            if name != partition_name:
                in_names.append(name)
        elif alloc.kind == "ExternalOutput":
            out_names.append(name)
            shape = tuple(alloc.tensor_shape)
            dtype = mybir.dt.np(alloc.dtype)
            out_avals.append(jax.core.ShapedArray(shape, dtype))
            zero_outs.append(np.zeros(shape, dtype))
    all_in_names = list(in_names) + out_names
    if partition_name is not None:
        all_in_names.append(partition_name)

    def _body(*args):
        operands = list(args)
        if partition_name is not None:
            operands.append(partition_id_tensor())
        outs = _bass_exec_p.bind(
            *operands,
            out_avals=tuple(out_avals),
            in_names=tuple(all_in_names),
            out_names=tuple(out_names),
            lowering_input_output_aliases=(),
            sim_require_finite=True,
            sim_require_nnan=True,
            nc=nc,
        )
        return tuple(outs)

    devices = jax.devices()[:N_CORES]
    assert len(devices) == N_CORES, f"need {N_CORES} devices, got {devices}"
    mesh = Mesh(np.asarray(devices), ("core",))
    nshard = NamedSharding(mesh, PartitionSpec("core"))
    n_in = len(in_names) + len(out_names)
    jf = jax.jit(
        shard_map(_body, mesh=mesh,
                  in_specs=(PartitionSpec("core"),) * n_in,
                  out_specs=(PartitionSpec("core"),) * len(out_names),
                  check_rep=False),
        keep_unused=True,
    )
    dev_zero = [
        jax.device_put(
            np.zeros((N_CORES * z.shape[0], *z.shape[1:]), z.dtype), nshard)
        for z in zero_outs
    ]

    def run(in_maps):
        concat_in = [
            np.concatenate([np.asarray(in_maps[c][nm])
                            for c in range(N_CORES)], axis=0)
            for nm in in_names
        ]
        dev_in = [jax.device_put(a, nshard) for a in concat_in]
        outs = jf(*dev_in, *dev_zero)
        return {
            name: np.asarray(outs[i]) for i, name in enumerate(out_names)
        }

    return run


def kernel(x, freqs_cis, wq, wk, wv, wo):
    if "nc" not in _cache:
        _cache["nc"] = _build_program()
    if "run" not in _cache:
        _cache["run"] = _make_runner(_cache["nc"])

    in_maps = _prep_inputs(
        np.asarray(x), np.asarray(freqs_cis), np.asarray(wq),
        np.asarray(wk), np.asarray(wv), np.asarray(wo))
    outs = _cache["run"](in_maps)
    pys = outs["py"].astype(np.float32).reshape(N_CORES, T, D)

    out = np.empty((B, T, D), dtype=np.float32)
    for b in range(B):
        acc = pys[b * 4]
        for g in range(1, 4):
            acc = acc + pys[b * 4 + g]
        out[b] = acc
    return out
